# revision 16
# baseline (speedup 1.0000x reference)
"""Trainium2 Bass kernel for a batch-as-time RNN-with-softmax-head problem.

Reference semantics (per step t over the leading axis of `input`):
    tmp      = concat([x_t, state], axis=1)            # (W, IN_F+HID)
    hidden   = tanh(tmp @ W1 + b1)                     # (W, HID)
    out_t    = softmax(hidden @ W2 + b2, axis=1)       # (W, NCLS)
    state    = hidden
Returns (out, final_state).

Sharding: the recurrence is row-wise independent over W (=2048), so rows are
split across the 8 NeuronCores (256 rows each) with zero communication.

On-chip layout: the hidden state is kept TRANSPOSED in SBUF as hT[hid, w]
(128 partitions x (4, 256)), which makes it directly usable as:
  - matmul rhs for the recurrent GEMM  (hidden_new^T = W1h^T @ hidden^T), and
  - matmul lhsT for the output GEMM    (logits = (hidden^T)^T @ W2).
x_t is DMA'd in natural layout and transposed on the tensor engine
(8 128x128 blocks per step). Biases are accumulated into PSUM with rank-1
(K=1) matmuls (ones x b2 for the logits; b1 x ones for the hidden), so both
tanh and exp run as plain activations.

mm_mode "f32r" uses the PE's FP32R format (tf32-like: 1s+8e+11m stored in the
high 20 bits) for matmul operands — 4x the fp32 matmul rate at free dim >=256.
The BIR verifier requires fp32r matmul inputs to be PRODUCED as fp32r, so the
operand tiles are declared float32r and the producing op (DVE copy / ACT tanh)
performs the rounding cast.

Scheduling constraint: a 4-byte-dtype Matmult carries its weight load in a
fused LDWEIGHTS and can encode only ONE sync-wait command. The per-iteration
emission order [W2(t-1)+softmax(t-1)] [W1(t)] [tanh(t)] [transpose x(t+1)]
plus the pool buffer counts below are arranged so every matmul needs at most
one new semaphore wait (all other deps are already covered by an earlier wait
on the same engine's semaphore at a higher tick).
"""

import os
from contextlib import ExitStack

import numpy as np

import concourse.bass as bass
import concourse.tile as tile
from concourse import bacc, masks, mybir
from concourse.bass_utils import run_bass_kernel_spmd

AF = mybir.ActivationFunctionType
DT = mybir.dt
AX = mybir.AxisListType

BATCH, W, IN_F, HID, NCLS = 32, 2048, 512, 512, 512
NCORES = 8
WS = W // NCORES       # 256 rows of the recurrence per core
WT = WS // 128         # 2 w-tiles per core
KI = IN_F // 128       # 4 contraction tiles for the x part
KH = HID // 128        # 4 contraction tiles for the h part
MH = HID // 128        # 4 output tiles of hidden

# matmul operand dtype: "f32" (exact, 4 cyc/row) or "f32r" (1 cyc/row at N>=256)
MM_MODE = os.environ.get("KERNEL_MM", "f32r")


def _emit(ctx: ExitStack, tc: "tile.TileContext", x, h0, w1, b1, w2, b2, out, hfin,
          mm_mode: str):
    nc = tc.nc
    f32 = DT.float32
    mdt = {"f32r": DT.float32r, "bf16": DT.bfloat16}.get(mm_mode, f32)
    # bias rank-1 matmuls in the same dtype as the main GEMMs: K=1 fp32r
    # matmuls measured 307-426 ns each (LDWEIGHTS-bound), bf16 hides the load
    bdt = mdt

    const = ctx.enter_context(tc.tile_pool(name="const", bufs=1))
    xpool = ctx.enter_context(tc.tile_pool(name="xp", bufs=3))
    xtpool = ctx.enter_context(tc.tile_pool(name="xtp", bufs=2))
    hpool = ctx.enter_context(tc.tile_pool(name="hp", bufs=2))
    epool = ctx.enter_context(tc.tile_pool(name="ep", bufs=2))
    opool = ctx.enter_context(tc.tile_pool(name="op", bufs=2))
    spool = ctx.enter_context(tc.tile_pool(name="sp", bufs=4))
    # PSUM budget (8 banks): psl 2x2 + psh 1x2 + pst 2x1 = 8
    psh = ctx.enter_context(tc.tile_pool(name="psh", bufs=1, space="PSUM"))
    psl = ctx.enter_context(tc.tile_pool(name="psl", bufs=2, space="PSUM"))
    pst = ctx.enter_context(tc.tile_pool(name="pst", bufs=2, space="PSUM"))

    # ---- constants ----
    ident = const.tile([128, 128], f32)
    masks.make_identity(nc, ident[:])

    def load_const(name, shape, dram_ap, dt):
        """DMA a weight into SBUF; round to the matmul dtype via a DVE copy."""
        st = const.tile(shape, f32, name=name + "_st", tag=name + "_st")
        nc.sync.dma_start(st[:], dram_ap)
        if dt == f32:
            return st
        t = const.tile(shape, dt, name=name, tag=name)
        nc.vector.tensor_copy(t[:], st[:])
        return t

    w1_sb = load_const("w1s", [128, KI + KH, HID],
                       w1.rearrange("(k p) h -> p k h", p=128), mdt)
    w2_sb = load_const("w2s", [128, KH, NCLS],
                       w2.rearrange("(k p) n -> p k n", p=128), mdt)
    b2_sb = load_const("b2s", [1, NCLS], b2.rearrange("(a n) -> a n", a=1), bdt)
    b1_sb = load_const("b1s", [1, HID], b1.rearrange("(a n) -> a n", a=1), bdt)
    ones_st = const.tile([1, WS], f32)
    nc.gpsimd.memset(ones_st[:], 1.0)
    if bdt != f32:
        ones_sb = const.tile([1, WS], bdt)
        nc.vector.tensor_copy(ones_sb[:], ones_st[:])
    else:
        ones_sb = ones_st

    x_r = x.rearrange("t (j p) f -> t p j f", p=128)       # [32][128, 2, 512]
    out_r = out.rearrange("t (j p) n -> t p j n", p=128)
    h0_r = h0.rearrange("(j p) h -> p j h", p=128)
    hfin_r = hfin.rearrange("(j p) h -> p j h", p=128)

    def load_x(t):
        xs = xpool.tile([128, WT, IN_F], f32, tag="xs", name=f"xs{t}")
        nc.sync.dma_start(xs[:], x_r[t])
        return xs

    def prep_x(xs, t):
        """In bf16 mode, cast x on the scalar engine before the PE transpose
        (bf16 transposes run at 1 cyc/row and their weight-load uses FWL)."""
        if mdt != DT.bfloat16:
            return xs[:]
        xc = xpool.tile([128, WT, IN_F], mdt, tag="xc", bufs=2, name=f"xc{t}")
        nc.scalar.copy(xc[:], xs[:])
        return xc[:]

    def transpose_in(nat_sb, dest, t, pool):
        """(w, feat)-natural SBUF tile -> (feat, w) transposed `dest`.

        bf16: 128x128 blocks through the DMA transpose XBAR (2-byte only) —
        costs zero PE time. Otherwise: PE transposes into PSUM, two
        half-groups of 4, each evacuated (and cast to the matmul dtype) by
        one DVE copy so the copy of half A overlaps the transposes of half B.
        """
        tdt = nat_sb.dtype
        if tdt == DT.bfloat16:
            for k in range(KI):
                for j in range(WT):
                    nc.sync.dma_start(
                        dest[:, k, bass.ts(j, 128)],
                        nat_sb[:, j, bass.ts(k, 128)], transpose=True,
                    )
            return
        for g in range(2):
            ptr = pool.tile([128, 2, WS], tdt, tag="ptr", name=f"ptr{t}_{g}")
            for kk in range(2):
                k = g * 2 + kk
                for j in range(WT):
                    nc.tensor.transpose(
                        ptr[:, kk, bass.ts(j, 128)],
                        nat_sb[:, j, bass.ts(k, 128)], ident[:]
                    )
            nc.vector.tensor_copy(dest[:, g * 2:(g + 1) * 2, :], ptr[:])

    # ---- prologue ----
    h0_sb = xpool.tile([128, WT, HID], f32, tag="xs", name="h0s")
    nc.sync.dma_start(h0_sb[:], h0_r)
    x_sb = [load_x(0), load_x(1)]

    # sacrificial transpose: gives PE a single-wait instruction that
    # observes the identity's gpsimd production before any real transpose
    scr = psl.tile([128, WT, NCLS], f32, tag="pl", name="pl_scr")
    nc.tensor.transpose(scr[:, 0, 0:128], ident[:], ident[:])

    # initial state into transposed layout (scratch PSUM from the psl pool)
    h_prev = hpool.tile([128, KH, WS], mdt, tag="hT", name="hT_init")
    for g in range(2):
        for kk in range(2):
            k = g * 2 + kk
            for j in range(WT):
                nc.tensor.transpose(
                    scr[:, kk, bass.ts(j, 128)],
                    h0_sb[:, j, bass.ts(k, 128)], ident[:]
                )
        nc.vector.tensor_copy(h_prev[:, g * 2:(g + 1) * 2, :], scr[:, 0:2, 0:WS])

    xt_cur = xtpool.tile([128, KI, WS], mdt, tag="xt", name="xt0")
    transpose_in(prep_x(x_sb[0], 0), xt_cur, 0, pst)

    h_hist = {}
    last_ph = None

    def emit_w2_softmax(s, h_s):
        pl = psl.tile([128, WT, NCLS], f32, tag="pl", name=f"pl{s}")
        for j in range(WT):
            nc.tensor.matmul(pl[:, j, :], ones_sb[:, 0:128], b2_sb[:],
                             start=True, stop=False)
            for k in range(KH):
                nc.tensor.matmul(
                    pl[:, j, :],
                    h_s[:, k, bass.ts(j, 128)],
                    w2_sb[:, k, :],
                    start=False, stop=(k == KH - 1),
                )
        for j in range(WT):
            nmax = spool.tile([128, 1], f32, tag="nm", name=f"nm{s}_{j}")
            nc.vector.reduce_max(nmax[:], pl[:, j, :], axis=AX.X, negate=True)
            esum = spool.tile([128, 1], f32, tag="es", name=f"es{s}_{j}")
            ex = epool.tile([128, NCLS], f32, tag="ex", name=f"ex{s}_{j}")
            nc.scalar.activation(ex[:], pl[:, j, :], AF.Exp, bias=nmax[:],
                                 accum_out=esum[:])
            rcp = spool.tile([128, 1], f32, tag="rc", name=f"rc{s}_{j}")
            nc.vector.reciprocal(rcp[:], esum[:])
            ob = opool.tile([128, NCLS], f32, tag="ob", name=f"ob{s}_{j}")
            nc.vector.tensor_scalar_mul(ob[:], ex[:], rcp[:])
            nc.sync.dma_start(out_r[s][:, j], ob[:])

    # ---- main loop ----
    for t in range(BATCH):
        if t + 2 < BATCH:
            x_sb.append(load_x(t + 2))
        if t - 1 >= 0:
            emit_w2_softmax(t - 1, h_hist.pop(t - 1))
        # W1 GEMM: hidden_t^T = tanh(W1x^T @ x_t^T + W1h^T @ h_{t-1}^T + b1)
        ph = psh.tile([128, MH, WS], f32, tag="ph", name=f"ph{t}")
        for m in range(MH):
            for k in range(KI):
                nc.tensor.matmul(
                    ph[:, m, :],
                    w1_sb[:, k, bass.ts(m, 128)],
                    xt_cur[:, k, :],
                    start=(k == 0), stop=False,
                )
            for k in range(KH):
                nc.tensor.matmul(
                    ph[:, m, :],
                    w1_sb[:, KI + k, bass.ts(m, 128)],
                    h_prev[:, k, :],
                    start=False, stop=False,
                )
            nc.tensor.matmul(            # + b1 (rank-1: b1_seg x ones_row)
                ph[:, m, :],
                b1_sb[:, bass.ts(m, 128)],
                ones_sb[:],
                start=False, stop=True,
            )
        h_t = hpool.tile([128, KH, WS], mdt, tag="hT", name=f"hT{t}")
        nc.scalar.activation(h_t[:], ph[:], AF.Tanh)
        # transpose next step's x while this step's tail runs
        if t + 1 < BATCH:
            xt_nxt = xtpool.tile([128, KI, WS], mdt, tag="xt", name=f"xt{t+1}")
            transpose_in(prep_x(x_sb[t + 1], t + 1), xt_nxt, t + 1, pst)
            xt_cur = xt_nxt
        h_hist[t] = h_t
        h_prev = h_t
        last_ph = ph

    # ---- epilogue ----
    emit_w2_softmax(BATCH - 1, h_hist.pop(BATCH - 1))

    # final state: recompute tanh(psum_31) in plain fp32 and transpose back
    hf_sb = epool.tile([128, MH, WS], f32, tag="hf", name="hf")
    nc.scalar.activation(hf_sb[:], last_ph[:], AF.Tanh)
    fs = opool.tile([128, WT, HID], f32, tag="fs", name="fs")
    for j in range(WT):
        ptr = pst.tile([128, 2, WS], f32, tag="ptr", name=f"ptrf{j}")
        for k in range(KH):
            nc.tensor.transpose(
                ptr[:, k // 2, bass.ts(k % 2, 128)],
                hf_sb[:, k, bass.ts(j, 128)], ident[:]
            )
        nc.vector.tensor_copy(
            fs[:, j, :].rearrange("p (a b) -> p a b", a=2), ptr[:, :, 0:WS]
        )
    nc.sync.dma_start(hfin_r, fs[:])


def build_nc(mm_mode: str = MM_MODE) -> "bass.Bass":
    # Bacc (not raw Bass): its compile() legalizes semaphore waits — TRN2
    # instructions can encode at most one sync wait; extras are split into
    # event-semaphore instructions.
    nc = bacc.Bacc("TRN2", target_bir_lowering=False, debug=False)
    f32 = DT.float32
    x = nc.dram_tensor("x", [BATCH, WS, IN_F], f32, kind="ExternalInput").ap()
    h0 = nc.dram_tensor("h0", [WS, HID], f32, kind="ExternalInput").ap()
    w1 = nc.dram_tensor("w1", [IN_F + HID, HID], f32, kind="ExternalInput").ap()
    b1 = nc.dram_tensor("b1", [HID], f32, kind="ExternalInput").ap()
    w2 = nc.dram_tensor("w2", [HID, NCLS], f32, kind="ExternalInput").ap()
    b2 = nc.dram_tensor("b2", [NCLS], f32, kind="ExternalInput").ap()
    out = nc.dram_tensor("out", [BATCH, WS, NCLS], f32, kind="ExternalOutput").ap()
    hfin = nc.dram_tensor("hfin", [WS, HID], f32, kind="ExternalOutput").ap()
    with tile.TileContext(nc) as tc:
        with ExitStack() as ctx:
            _emit(ctx, tc, x, h0, w1, b1, w2, b2, out, hfin, mm_mode)
    nc.compile()
    return nc


def make_in_maps(inputs: dict) -> list[dict]:
    x = np.ascontiguousarray(inputs["input"], dtype=np.float32)
    h0 = np.ascontiguousarray(inputs["pre_state"], dtype=np.float32)
    w1 = np.ascontiguousarray(inputs["W1"], dtype=np.float32)
    b1 = np.ascontiguousarray(inputs["b1"], dtype=np.float32)
    w2 = np.ascontiguousarray(inputs["W2"], dtype=np.float32)
    b2 = np.ascontiguousarray(inputs["b2"], dtype=np.float32)
    maps = []
    for c in range(NCORES):
        sl = slice(c * WS, (c + 1) * WS)
        maps.append({
            "x": np.ascontiguousarray(x[:, sl, :]),
            "h0": np.ascontiguousarray(h0[sl]),
            "w1": w1, "b1": b1, "w2": w2, "b2": b2,
        })
    return maps


_NC_CACHE: dict = {}


def _get_nc(mm_mode: str):
    if mm_mode not in _NC_CACHE:
        _NC_CACHE[mm_mode] = build_nc(mm_mode)
    return _NC_CACHE[mm_mode]


def run(inputs: dict, trace: bool = False, mm_mode: str = MM_MODE):
    nc = _get_nc(mm_mode)
    res = run_bass_kernel_spmd(nc, make_in_maps(inputs),
                               core_ids=list(range(NCORES)), trace=trace)
    out = np.concatenate([r["out"] for r in res.results], axis=1)
    fin = np.concatenate([r["hfin"] for r in res.results], axis=0)
    return (out, fin), res


def kernel(**inputs):
    (out, fin), _ = run(inputs)
    return out, fin


# revision 19
# speedup vs baseline: 2.4685x; 2.4685x over previous
"""Trainium2 Bass kernel for a batch-as-time RNN-with-softmax-head problem.

Reference semantics (per step t over the leading axis of `input`):
    tmp      = concat([x_t, state], axis=1)            # (W, IN_F+HID)
    hidden   = tanh(tmp @ W1 + b1)                     # (W, HID)
    out_t    = softmax(hidden @ W2 + b2, axis=1)       # (W, NCLS)
    state    = hidden
Returns (out, final_state).

Sharding: the recurrence is row-wise independent over W (=2048), so rows are
split across the 8 NeuronCores (256 rows each) with zero communication.

On-chip layout: the hidden state is kept TRANSPOSED in SBUF as hT[hid, w]
(128 partitions x (4, 256)), which makes it directly usable as:
  - matmul rhs for the recurrent GEMM  (hidden_new^T = W1h^T @ hidden^T), and
  - matmul lhsT for the output GEMM    (logits = (hidden^T)^T @ W2).
x_t is DMA'd in natural layout and transposed on the tensor engine
(8 128x128 blocks per step). Biases are accumulated into PSUM with rank-1
(K=1) matmuls (ones x b2 for the logits; b1 x ones for the hidden), so both
tanh and exp run as plain activations.

mm_mode "f32r" uses the PE's FP32R format (tf32-like: 1s+8e+11m stored in the
high 20 bits) for matmul operands — 4x the fp32 matmul rate at free dim >=256.
The BIR verifier requires fp32r matmul inputs to be PRODUCED as fp32r, so the
operand tiles are declared float32r and the producing op (DVE copy / ACT tanh)
performs the rounding cast.

Scheduling constraint: a 4-byte-dtype Matmult carries its weight load in a
fused LDWEIGHTS and can encode only ONE sync-wait command. The per-iteration
emission order [W2(t-1)+softmax(t-1)] [W1(t)] [tanh(t)] [transpose x(t+1)]
plus the pool buffer counts below are arranged so every matmul needs at most
one new semaphore wait (all other deps are already covered by an earlier wait
on the same engine's semaphore at a higher tick).
"""

import os
from contextlib import ExitStack

import numpy as np

import concourse.bass as bass
import concourse.tile as tile
from concourse import bacc, masks, mybir
from concourse.bass_utils import run_bass_kernel_spmd

AF = mybir.ActivationFunctionType
DT = mybir.dt
AX = mybir.AxisListType

BATCH, W, IN_F, HID, NCLS = 32, 2048, 512, 512, 512
NCORES = 8
WS = W // NCORES       # 256 rows of the recurrence per core
WT = WS // 128         # 2 w-tiles per core
KI = IN_F // 128       # 4 contraction tiles for the x part
KH = HID // 128        # 4 contraction tiles for the h part
MH = HID // 128        # 4 output tiles of hidden

# matmul operand dtype: "f32" (exact, 4 cyc/row) or "f32r" (1 cyc/row at N>=256)
MM_MODE = os.environ.get("KERNEL_MM", "f32r")


def _emit(ctx: ExitStack, tc: "tile.TileContext", x, h0, w1, b1, w2, b2, out, hfin,
          mm_mode: str):
    nc = tc.nc
    f32 = DT.float32
    mdt = {"f32r": DT.float32r, "bf16": DT.bfloat16}.get(mm_mode, f32)
    # bias rank-1 matmuls in the same dtype as the main GEMMs: K=1 fp32r
    # matmuls measured 307-426 ns each (LDWEIGHTS-bound), bf16 hides the load
    bdt = mdt

    const = ctx.enter_context(tc.tile_pool(name="const", bufs=1))
    xpool = ctx.enter_context(tc.tile_pool(name="xp", bufs=3))
    xtpool = ctx.enter_context(tc.tile_pool(name="xtp", bufs=2))
    hpool = ctx.enter_context(tc.tile_pool(name="hp", bufs=2))
    epool = ctx.enter_context(tc.tile_pool(name="ep", bufs=2))
    opool = ctx.enter_context(tc.tile_pool(name="op", bufs=2))
    spool = ctx.enter_context(tc.tile_pool(name="sp", bufs=4))
    # PSUM budget (8 banks): psl 2x2 + psh 1x2 + pst 2x1 = 8
    psh = ctx.enter_context(tc.tile_pool(name="psh", bufs=1, space="PSUM"))
    psl = ctx.enter_context(tc.tile_pool(name="psl", bufs=2, space="PSUM"))
    pst = ctx.enter_context(tc.tile_pool(name="pst", bufs=2, space="PSUM"))

    # ---- constants ----
    ident = const.tile([128, 128], f32)
    masks.make_identity(nc, ident[:])
    if mdt == DT.bfloat16:
        ident_t = const.tile([128, 128], mdt)   # for bf16 transposes
        masks.make_identity(nc, ident_t[:])
    else:
        ident_t = ident

    def load_const(name, shape, dram_ap, dt):
        """DMA a weight into SBUF; round to the matmul dtype via a DVE copy."""
        st = const.tile(shape, f32, name=name + "_st", tag=name + "_st")
        nc.sync.dma_start(st[:], dram_ap)
        if dt == f32:
            return st
        t = const.tile(shape, dt, name=name, tag=name)
        nc.vector.tensor_copy(t[:], st[:])
        return t

    w1_sb = load_const("w1s", [128, KI + KH, HID],
                       w1.rearrange("(k p) h -> p k h", p=128), mdt)
    w2_sb = load_const("w2s", [128, KH, NCLS],
                       w2.rearrange("(k p) n -> p k n", p=128), mdt)
    b2_sb = load_const("b2s", [1, NCLS], b2.rearrange("(a n) -> a n", a=1), bdt)
    b1_sb = load_const("b1s", [1, HID], b1.rearrange("(a n) -> a n", a=1), bdt)
    ones_st = const.tile([1, WS], f32)
    nc.gpsimd.memset(ones_st[:], 1.0)
    if bdt != f32:
        ones_sb = const.tile([1, WS], bdt)
        nc.vector.tensor_copy(ones_sb[:], ones_st[:])
    else:
        ones_sb = ones_st

    x_r = x.rearrange("t (j p) f -> t p j f", p=128)       # [32][128, 2, 512]
    out_r = out.rearrange("t (j p) n -> t p j n", p=128)
    h0_r = h0.rearrange("(j p) h -> p j h", p=128)
    hfin_r = hfin.rearrange("(j p) h -> p j h", p=128)

    def load_x(t):
        xs = xpool.tile([128, WT, IN_F], f32, tag="xs", name=f"xs{t}")
        nc.sync.dma_start(xs[:], x_r[t])
        return xs

    def prep_x(xs, t):
        """In bf16 mode, cast x on the scalar engine before the PE transpose
        (bf16 transposes run at 1 cyc/row and their weight-load uses FWL)."""
        if mdt != DT.bfloat16:
            return xs[:]
        xc = xpool.tile([128, WT, IN_F], mdt, tag="xc", bufs=2, name=f"xc{t}")
        nc.scalar.copy(xc[:], xs[:])
        return xc[:]

    def transpose_in(nat_sb, dest, t, pool):
        """(w, feat)-natural SBUF tile -> (feat, w) transposed `dest`.

        PE transposes into PSUM, two half-groups of 4, each evacuated (and
        cast to the matmul dtype) by one DVE copy so the copy of half A
        overlaps the transposes of half B. (DMA-transpose XBAR was tried for
        bf16 and regressed 2.4x — DMATranspose/DMACopy xbar-mode transitions
        serialize the HWDGE queues.)"""
        tdt = nat_sb.dtype
        idn = ident_t if tdt == DT.bfloat16 else ident
        for g in range(2):
            ptr = pool.tile([128, 2, WS], tdt, tag="ptr", name=f"ptr{t}_{g}")
            for kk in range(2):
                k = g * 2 + kk
                for j in range(WT):
                    nc.tensor.transpose(
                        ptr[:, kk, bass.ts(j, 128)],
                        nat_sb[:, j, bass.ts(k, 128)], idn[:]
                    )
            nc.vector.tensor_copy(dest[:, g * 2:(g + 1) * 2, :], ptr[:])

    # ---- prologue ----
    h0_sb = xpool.tile([128, WT, HID], f32, tag="xs", name="h0s")
    nc.sync.dma_start(h0_sb[:], h0_r)
    x_sb = [load_x(0), load_x(1)]

    # sacrificial transpose: gives PE a single-wait instruction that
    # observes the identity's gpsimd production before any real transpose
    scr = psl.tile([128, WT, NCLS], f32, tag="pl", name="pl_scr")
    nc.tensor.transpose(scr[:, 0, 0:128], ident[:], ident[:])

    # initial state into transposed layout (scratch PSUM from the psl pool)
    h_prev = hpool.tile([128, KH, WS], mdt, tag="hT", name="hT_init")
    for g in range(2):
        for kk in range(2):
            k = g * 2 + kk
            for j in range(WT):
                nc.tensor.transpose(
                    scr[:, kk, bass.ts(j, 128)],
                    h0_sb[:, j, bass.ts(k, 128)], ident[:]
                )
        nc.vector.tensor_copy(h_prev[:, g * 2:(g + 1) * 2, :], scr[:, 0:2, 0:WS])

    xt_cur = xtpool.tile([128, KI, WS], mdt, tag="xt", name="xt0")
    transpose_in(prep_x(x_sb[0], 0), xt_cur, 0, pst)

    h_hist = {}
    last_ph = None

    def emit_w2_softmax(s, h_s):
        pl = psl.tile([128, WT, NCLS], f32, tag="pl", name=f"pl{s}")
        for j in range(WT):
            nc.tensor.matmul(pl[:, j, :], ones_sb[:, 0:128], b2_sb[:],
                             start=True, stop=False)
            for k in range(KH):
                nc.tensor.matmul(
                    pl[:, j, :],
                    h_s[:, k, bass.ts(j, 128)],
                    w2_sb[:, k, :],
                    start=False, stop=(k == KH - 1),
                )
        for j in range(WT):
            nmax = spool.tile([128, 1], f32, tag="nm", name=f"nm{s}_{j}")
            nc.vector.reduce_max(nmax[:], pl[:, j, :], axis=AX.X, negate=True)
            esum = spool.tile([128, 1], f32, tag="es", name=f"es{s}_{j}")
            ex = epool.tile([128, NCLS], f32, tag="ex", name=f"ex{s}_{j}")
            nc.scalar.activation(ex[:], pl[:, j, :], AF.Exp, bias=nmax[:],
                                 accum_out=esum[:])
            rcp = spool.tile([128, 1], f32, tag="rc", name=f"rc{s}_{j}")
            nc.vector.reciprocal(rcp[:], esum[:])
            ob = opool.tile([128, NCLS], f32, tag="ob", name=f"ob{s}_{j}")
            nc.vector.tensor_scalar_mul(ob[:], ex[:], rcp[:])
            nc.sync.dma_start(out_r[s][:, j], ob[:])

    # ---- main loop ----
    for t in range(BATCH):
        if t + 2 < BATCH:
            x_sb.append(load_x(t + 2))
        if t - 1 >= 0:
            emit_w2_softmax(t - 1, h_hist.pop(t - 1))
        # W1 GEMM: hidden_t^T = tanh(W1x^T @ x_t^T + W1h^T @ h_{t-1}^T + b1)
        ph = psh.tile([128, MH, WS], f32, tag="ph", name=f"ph{t}")
        for m in range(MH):
            for k in range(KI):
                nc.tensor.matmul(
                    ph[:, m, :],
                    w1_sb[:, k, bass.ts(m, 128)],
                    xt_cur[:, k, :],
                    start=(k == 0), stop=False,
                )
            for k in range(KH):
                nc.tensor.matmul(
                    ph[:, m, :],
                    w1_sb[:, KI + k, bass.ts(m, 128)],
                    h_prev[:, k, :],
                    start=False, stop=False,
                )
            nc.tensor.matmul(            # + b1 (rank-1: b1_seg x ones_row)
                ph[:, m, :],
                b1_sb[:, bass.ts(m, 128)],
                ones_sb[:],
                start=False, stop=True,
            )
        h_t = hpool.tile([128, KH, WS], mdt, tag="hT", name=f"hT{t}")
        nc.scalar.activation(h_t[:], ph[:], AF.Tanh)
        # transpose next step's x while this step's tail runs
        if t + 1 < BATCH:
            xt_nxt = xtpool.tile([128, KI, WS], mdt, tag="xt", name=f"xt{t+1}")
            transpose_in(prep_x(x_sb[t + 1], t + 1), xt_nxt, t + 1, pst)
            xt_cur = xt_nxt
        h_hist[t] = h_t
        h_prev = h_t
        last_ph = ph

    # ---- epilogue ----
    emit_w2_softmax(BATCH - 1, h_hist.pop(BATCH - 1))

    # final state: recompute tanh(psum_31) in plain fp32 and transpose back
    hf_sb = epool.tile([128, MH, WS], f32, tag="hf", name="hf")
    nc.scalar.activation(hf_sb[:], last_ph[:], AF.Tanh)
    fs = opool.tile([128, WT, HID], f32, tag="fs", name="fs")
    for j in range(WT):
        ptr = pst.tile([128, 2, WS], f32, tag="ptr", name=f"ptrf{j}")
        for k in range(KH):
            nc.tensor.transpose(
                ptr[:, k // 2, bass.ts(k % 2, 128)],
                hf_sb[:, k, bass.ts(j, 128)], ident[:]
            )
        nc.vector.tensor_copy(
            fs[:, j, :].rearrange("p (a b) -> p a b", a=2), ptr[:, :, 0:WS]
        )
    nc.sync.dma_start(hfin_r, fs[:])


def build_nc(mm_mode: str = MM_MODE) -> "bass.Bass":
    # Bacc (not raw Bass): its compile() legalizes semaphore waits — TRN2
    # instructions can encode at most one sync wait; extras are split into
    # event-semaphore instructions.
    nc = bacc.Bacc("TRN2", target_bir_lowering=False, debug=False)
    f32 = DT.float32
    x = nc.dram_tensor("x", [BATCH, WS, IN_F], f32, kind="ExternalInput").ap()
    h0 = nc.dram_tensor("h0", [WS, HID], f32, kind="ExternalInput").ap()
    w1 = nc.dram_tensor("w1", [IN_F + HID, HID], f32, kind="ExternalInput").ap()
    b1 = nc.dram_tensor("b1", [HID], f32, kind="ExternalInput").ap()
    w2 = nc.dram_tensor("w2", [HID, NCLS], f32, kind="ExternalInput").ap()
    b2 = nc.dram_tensor("b2", [NCLS], f32, kind="ExternalInput").ap()
    out = nc.dram_tensor("out", [BATCH, WS, NCLS], f32, kind="ExternalOutput").ap()
    hfin = nc.dram_tensor("hfin", [WS, HID], f32, kind="ExternalOutput").ap()
    with tile.TileContext(nc) as tc:
        with ExitStack() as ctx:
            _emit(ctx, tc, x, h0, w1, b1, w2, b2, out, hfin, mm_mode)
    nc.compile()
    return nc


def make_in_maps(inputs: dict) -> list[dict]:
    x = np.ascontiguousarray(inputs["input"], dtype=np.float32)
    h0 = np.ascontiguousarray(inputs["pre_state"], dtype=np.float32)
    w1 = np.ascontiguousarray(inputs["W1"], dtype=np.float32)
    b1 = np.ascontiguousarray(inputs["b1"], dtype=np.float32)
    w2 = np.ascontiguousarray(inputs["W2"], dtype=np.float32)
    b2 = np.ascontiguousarray(inputs["b2"], dtype=np.float32)
    maps = []
    for c in range(NCORES):
        sl = slice(c * WS, (c + 1) * WS)
        maps.append({
            "x": np.ascontiguousarray(x[:, sl, :]),
            "h0": np.ascontiguousarray(h0[sl]),
            "w1": w1, "b1": b1, "w2": w2, "b2": b2,
        })
    return maps


_NC_CACHE: dict = {}


def _get_nc(mm_mode: str):
    if mm_mode not in _NC_CACHE:
        _NC_CACHE[mm_mode] = build_nc(mm_mode)
    return _NC_CACHE[mm_mode]


def run(inputs: dict, trace: bool = False, mm_mode: str = MM_MODE):
    nc = _get_nc(mm_mode)
    res = run_bass_kernel_spmd(nc, make_in_maps(inputs),
                               core_ids=list(range(NCORES)), trace=trace)
    out = np.concatenate([r["out"] for r in res.results], axis=1)
    fin = np.concatenate([r["hfin"] for r in res.results], axis=0)
    return (out, fin), res


def kernel(**inputs):
    (out, fin), _ = run(inputs)
    return out, fin
